# revision 54
# baseline (speedup 1.0000x reference)
"""Causal self-attention kernel for 8 trn2 NeuronCores.

Sharding: core c handles batch b = c // 4 and local head group hg = c % 4
(4 of the 16 heads). Tensor-parallel over heads for kqv / attention and
row-parallel for the output projection; the 4 per-batch partial projections
(bf16) are summed on the host, where the bias is also added.

Inputs are pre-tiled on the host into [128, *] SBUF-ready layouts so every
load is a large multi-queue DMA. Feature tiles are packed [k01,q01,k23,q23]
so the first DMA chunk covers exactly what the first attention blocks need.

Device kernel (per core): one flat software-pipelined emitter over all
(window, head-pair) segments:
  - each pair's O-matmuls are emitted one pair late, and each segment's
    O-flush + normalize is emitted after the NEXT segment's first S/exp, so
    neither the PE queue nor ScalarE drains at hp/window boundaries (a
    drained PE queue also defeats the background LDWEIGHTS prefetch, which
    is what lets back-to-back matmuls sustain ~216ns/512cols)
  - dense work (later windows' k/q/v projections, earlier windows' output
    projections) lives in one global filler queue, popped between exp
    emissions proportionally to the remaining blocks; force-points drain a
    unit group before the instruction that reads its output is emitted
  - fp8e4: kq for windows 1-3 uses DoubleRowSwInterleave with host-packed
    interleaved+reversed weights (contiguous fast weight load, full 2x fp8
    column rate); v projections and attention O-matmuls use DoubleRow pair
    views; window 0 kq/v and the output projection stay bf16 (fp8 noise on
    short-context rows / in the projection reaches the output unaveraged
    and blows the 2e-2 budget - measured, not theoretical)
  - exp strips: S^T psum -> ScalarE exp -> fp8 [j0:h0|h1 | j1:h0|h1] tile;
    diagonal blocks use one 3D-AP activation covering both heads' [cs:512]
    strips (saves the 2nd instruction's ~350-cycle overhead); causal masks
    via GpSimd affine_select; denominators via ones-columns in the O
    stationary (psum rows 64:128 accumulate sum(P) for free)
  - tail: window 3's oc_s[0] projection half (proj3_a) runs as filler in
    the hp=1 exp shadow; after the last normalize (ScalarE lsb copies + a
    gated bf16 keep-warm matmul), proj3_b folds the bf16 a-half into psum
    with an identity matmul and stages out via ScalarE/DVE copies, each
    128-row y strip DMA'd (bf16) as soon as it completes

Per tq window g of 512 (pairs m <= 2g+1, causally trimmed):
  S^T = k^T.T q^T   (K=64; the head pair runs concurrently on PE row
                     groups 0-63/64-127 via base_partition tile_position)
  P = exp(S^T/8)    no max subtraction (scores are O(1))
  O^T psum[0:64] += [v_2m v_2m+1] @ [P_2m P_2m+1]   (fp8 DoubleRow)
  normalize: copies stack both heads' denominators in one tile, one
             reciprocal_approx_fast covers both, then multiply -> oc bf16
  proj: y[:, window] = O_cat^T.T @ Wp^T -> bf16 -> DMA
"""

import numpy as np
import ml_dtypes

T = 2048
C = 1024
NH_LOCAL = 4
D = 64
TQW = 512  # tq window width
NGRP = T // TQW  # 4 tq windows

_nc_cache = {}


def _build_bass():
    import concourse.mybir as mybir
    import concourse.tile as tile
    from concourse import bacc

    f32 = mybir.dt.float32
    bf16 = mybir.dt.bfloat16
    f8 = mybir.dt.float8e4

    nc = bacc.Bacc(None, target_bir_lowering=False)
    # pre-tiled inputs: [128, packed free dim] (see _shard_inputs)
    # bf16 copies feed window 0 (short-context rows need the precision);
    # fp8 pair-interleaved copies feed windows 1-3 via DoubleRow matmuls
    xt_d = nc.dram_tensor("xt", [128, 4096], bf16, kind="ExternalInput")
    xtf_d = nc.dram_tensor("xtf", [128, 3 * 4096], f8, kind="ExternalInput")
    wqk_d = nc.dram_tensor("wqk", [128, 8 * 512], bf16, kind="ExternalInput")
    wqkf_d = nc.dram_tensor("wqkf", [128, 4096], f8, kind="ExternalInput")
    wv_d = nc.dram_tensor("wv", [128, 8 * 256], bf16, kind="ExternalInput")
    wvf_d = nc.dram_tensor("wvf", [128, 2048], f8, kind="ExternalInput")
    wp_d = nc.dram_tensor("wp", [128, 2 * C], bf16, kind="ExternalInput")
    eye_d = nc.dram_tensor("eye", [128, 128], bf16, kind="ExternalInput")
    y_d = nc.dram_tensor("y", [T, C], bf16, kind="ExternalOutput")

    # host packs wqk/wqkf feature tiles in order [k01, q01, k23, q23] so the
    # first DMA chunk covers everything attn(0) hp=0 needs; f is the semantic
    # tile index (0=k01, 1=k23, 2=q01, 3=q23) used by kq_s / attn
    FBLK = {0: 0, 2: 1, 1: 2, 3: 3}

    with tile.TileContext(nc) as tc:
        with (
            tc.tile_pool(name="persist", bufs=1) as pp,
            tc.tile_pool(name="mmp", bufs=2, space="PSUM") as mp,
            tc.tile_pool(name="spsum", bufs=2, space="PSUM") as sp,
            tc.tile_pool(name="opsum", bufs=1, space="PSUM") as op,
            tc.tile_pool(name="ptp", bufs=4) as ptp,
            tc.tile_pool(name="rp", bufs=4) as rp,
            tc.tile_pool(name="ysb", bufs=3) as ysb,
        ):
            xt_a = pp.tile([128, 2048], bf16, tag="xta", name="xta")
            xt_b = pp.tile([128, 2048], bf16, tag="xtb", name="xtb")
            xtf_s = pp.tile([128, 3 * 4096], f8, tag="xtf", name="xtf")
            wqk_a = pp.tile([128, 2048], bf16, tag="wqka", name="wqka")
            wqk_b = pp.tile([128, 2048], bf16, tag="wqkb", name="wqkb")
            wqkf_s = pp.tile([128, 4096], f8, tag="wqkf", name="wqkf")
            wv_s = pp.tile([128, 8 * 256], bf16, tag="wv", name="wv")
            wvf_s = pp.tile([128, 2048], f8, tag="wvf", name="wvf")
            wp_s = pp.tile([128, 2 * C], bf16, tag="wp", name="wp")
            eye_s = pp.tile([128, 128], bf16, tag="eye", name="eye")
            kq_s = [pp.tile([128, T], bf16, tag=f"kq{f}", name=f"kq{f}") for f in range(4)]
            v_s = pp.tile([128, 4 * T], f8, tag="vall", name="vall")
            oc_s = [pp.tile([128, T], bf16, tag=f"oc{p}", name=f"oc{p}") for p in range(2)]
            wu_s = pp.tile([128, 512], bf16, tag="wu", name="wu")
            # bf16 copy of tk chunks j=0,1 (v | ones): short-context queries
            # (t<256) see little averaging, too noisy for the fp8 path
            vb_s = pp.tile([128, 1024], bf16, tag="vb", name="vb")
            # window-3 proj strips held in SBUF across the hp=0/hp=1 split
            ys3_s = [
                pp.tile([128, C], bf16, tag=f"ys3_{i}", name=f"ys3_{i}")
                for i in range(4)
            ]

            # slicing helpers for the packed layouts
            def xt_w(c):  # bf16 moving operand, window 0, contraction chunk c
                t = xt_a if c < 4 else xt_b
                return t[:, 512 * (c % 4) : 512 * (c % 4 + 1)]

            def xt_j(c, j):  # bf16 stationary for v chunks 0-3
                t = xt_a if c < 4 else xt_b
                o = 512 * (c % 4) + 128 * (j % 4)
                return t[:, o : o + 128]

            def wqk_blk(b):  # bf16 feature block b (packed order)
                t = wqk_a if b < 2 else wqk_b
                return t[:, 1024 * (b % 2) : 1024 * (b % 2 + 1)]

            def xtf_w(cp, g):  # fp8 [p,2,512] rhs pair view, windows 1-3
                o = 4096 * (g - 1) + 1024 * cp
                return xtf_s[:, o : o + 1024].rearrange(
                    "p (two x) -> p two x", two=2
                )

            def xtf_j(cp, j):  # fp8 [p,2,128] stationary pair view, chunks 4-15
                o = 4096 * (j // 4 - 1) + 1024 * cp
                t0 = 128 * (j % 4)
                return xtf_s[:, o : o + 1024].rearrange(
                    "p (two x) -> p two x", two=2
                )[:, :, t0 : t0 + 128]

            # ---- HAM warmup: PE busy from boot so real matmuls run warm ----
            # short: just enough to cover the first DMA chunk's latency; the
            # first kq chains continue the warm streak
            nc.gpsimd.memset(wu_s[:], 0.03125)
            for _ in range(12):
                wacc = mp.tile([128, 512], f32, tag="mm", name="warm")
                nc.tensor.matmul(wacc[:], wu_s[:, 0:128], wu_s[:], start=True, stop=True)

            # ones blocks for the denominator trick (v regions overwritten
            # later); contiguous memset on GpSimd - off Vector's critical path
            nc.gpsimd.memset(v_s[:], 1.0)
            nc.gpsimd.memset(vb_s[:], 1.0)

            # input DMAs: few large issues ordered by first use (a single
            # dma_start is split across all 16 queues by the framework)
            nc.sync.dma_start(wqk_a[:], wqk_d[:, 0:2048])
            nc.sync.dma_start(xt_a[:], xt_d[:, 0:2048])
            nc.sync.dma_start(xt_b[:], xt_d[:, 2048:4096])
            nc.sync.dma_start(wv_s[:], wv_d[:])
            nc.sync.dma_start(wqk_b[:], wqk_d[:, 2048:4096])
            nc.sync.dma_start(wqkf_s[:], wqkf_d[:])
            nc.sync.dma_start(xtf_s[:, 0:4096], xtf_d[:, 0:4096])
            nc.sync.dma_start(wvf_s[:], wvf_d[:])
            nc.sync.dma_start(wp_s[:], wp_d[:])
            nc.sync.dma_start(xtf_s[:, 4096:12288], xtf_d[:, 4096:12288])
            nc.sync.dma_start(eye_s[:], eye_d[:])

            # ---- dense work generators (emitted one instruction at a time) ----
            def kq_units(g, fs=(0, 1, 2, 3)):
                units = []
                for f in fs:
                    b = FBLK[f]
                    st = {}
                    if g == 0:
                        for c in range(8):
                            def mm(b=b, c=c, st=st):
                                if c == 0:
                                    st["acc"] = mp.tile([128, 512], f32, tag="mm", name="mmkq")
                                nc.tensor.matmul(
                                    st["acc"][:],
                                    wqk_blk(b)[:, 128 * c : 128 * (c + 1)],
                                    xt_w(c),
                                    start=(c == 0),
                                    stop=(c == 7),
                                )
                            units.append(mm)
                    else:
                        for cp_ in range(4):
                            def mm(b=b, cp_=cp_, g=g, st=st):
                                if cp_ == 0:
                                    st["acc"] = mp.tile([128, 512], f32, tag="mm", name="mmkq")
                                # SwInterleave: wqkf is host-packed with the
                                # A/B contraction pair interleaved per feature
                                # column (reversed), so LDWEIGHTS is one
                                # contiguous fast load and the fp8 matmul
                                # sustains its full 2x column rate
                                nc.tensor.matmul(
                                    st["acc"][:],
                                    wqkf_s[
                                        :, 1024 * b + 256 * cp_ : 1024 * b + 256 * (cp_ + 1)
                                    ],
                                    xtf_w(cp_, g),
                                    start=(cp_ == 0),
                                    stop=(cp_ == 3),
                                    perf_mode=mybir.MatmulPerfMode.DoubleRowSwInterleave,
                                )
                            units.append(mm)
                    def cp(f=f, g=g, st=st):
                        nc.vector.tensor_copy(
                            kq_s[f][:, TQW * g : TQW * (g + 1)], st["acc"][:]
                        )
                    units.append(cp)
                return units

            def v_units(g, js=None):
                units = []
                for j in js if js is not None else range(4 * g, 4 * g + 4):
                    st = {}
                    if g == 0:
                        for c in range(8):
                            def mm(j=j, c=c, st=st):
                                if c == 0:
                                    st["acc"] = mp.tile([128, 512], f32, tag="mm", name="mmv")
                                nc.tensor.matmul(
                                    st["acc"][:, :256],
                                    xt_j(c, j),
                                    wv_s[:, 256 * c : 256 * (c + 1)],
                                    start=(c == 0),
                                    stop=(c == 7),
                                )
                            units.append(mm)
                    else:
                        for cp_ in range(4):
                            def mm(j=j, cp_=cp_, st=st):
                                if cp_ == 0:
                                    st["acc"] = mp.tile([128, 512], f32, tag="mm", name="mmv")
                                nc.tensor.matmul(
                                    st["acc"][:, :256],
                                    xtf_j(cp_, j),
                                    wvf_s[:, 512 * cp_ : 512 * (cp_ + 1)].rearrange(
                                        "p (two x) -> p two x", two=2
                                    ),
                                    start=(cp_ == 0),
                                    stop=(cp_ == 3),
                                    perf_mode=mybir.MatmulPerfMode.DoubleRow,
                                )
                            units.append(mm)
                    def cp(j=j, st=st):
                        nc.vector.tensor_copy(
                            v_s[:, 512 * j : 512 * j + 512].rearrange(
                                "p (h x) -> p h x", h=4
                            )[:, :, 0:64],
                            st["acc"][:, 0:256].rearrange("p (h x) -> p h x", h=4),
                        )
                        if j < 2:
                            nc.vector.tensor_copy(
                                vb_s[:, 512 * j : 512 * (j + 1)].rearrange(
                                    "p (h x) -> p h x", h=4
                                )[:, :, 0:64],
                                st["acc"][:, 0:256].rearrange("p (h x) -> p h x", h=4),
                            )
                    units.append(cp)
                return units

            def proj_units(g):
                # windows 0..2: full projection, y strips DMA'd in pairs
                units = []
                for pi in range(2):
                    i0 = 4 * g + 2 * pi
                    st = {}
                    def alloc(st=st):
                        st["ys"] = ysb.tile([128, 2 * C], bf16, tag="ys", name="ys")
                    units.append(alloc)
                    for k in range(2):
                        i = i0 + k
                        for u in range(2):
                            for ci in range(2):
                                def mm(i=i, u=u, ci=ci, st=st):
                                    if ci == 0:
                                        st["acc"] = mp.tile([128, 512], f32, tag="mm", name="mmy")
                                    nc.tensor.matmul(
                                        st["acc"][:],
                                        oc_s[ci][:, 128 * i : 128 * (i + 1)],
                                        wp_s[:, 1024 * ci + 512 * u : 1024 * ci + 512 * (u + 1)],
                                        start=(ci == 0),
                                        stop=(ci == 1),
                                    )
                                units.append(mm)
                            def cp(k=k, u=u, st=st):
                                nc.vector.tensor_copy(
                                    st["ys"][:, 1024 * k + 512 * u : 1024 * k + 512 * (u + 1)],
                                    st["acc"][:],
                                )
                            units.append(cp)
                    def out(i0=i0, st=st):
                        nc.sync.dma_start(
                            y_d[128 * i0 : 128 * i0 + 256, :].rearrange(
                                "(two p) c -> p two c", two=2
                            ),
                            st["ys"][:].rearrange("p (two c) -> p two c", two=2),
                        )
                    units.append(out)
                return units

            def proj3_a_units():
                # window 3, oc_s[0] (head pair 0) half: filler for attn(3)
                # hp=1; partial y strips stream out early (bypass DMA)
                units = []
                for k, i in enumerate(range(12, 16)):
                    st = {}
                    for u in range(2):
                        def mm(i=i, u=u, st=st):
                            st["acc"] = mp.tile([128, 512], f32, tag="mm", name="mmy")
                            nc.tensor.matmul(
                                st["acc"][:],
                                oc_s[0][:, 128 * i : 128 * (i + 1)],
                                wp_s[:, 512 * u : 512 * (u + 1)],
                                start=True,
                                stop=True,
                            )
                        units.append(mm)
                        def cp(k=k, u=u, st=st):
                            nc.vector.tensor_copy(
                                ys3_s[k][:, 512 * u : 512 * (u + 1)], st["acc"][:]
                            )
                        units.append(cp)
                return units

            def proj3_b_units():
                # window 3, oc_s[1] half: the bf16 a-half is folded into the
                # psum with an identity matmul (PE), then the psum is staged
                # out with one ScalarE copy and one DVE copy per strip so no
                # single engine serializes the tail
                units = []
                for k, i in enumerate(range(12, 16)):
                    st = {}
                    def alloc(st=st):
                        st["yo"] = ysb.tile([128, C], bf16, tag="yo", name="yo")
                    units.append(alloc)
                    for u in range(2):
                        def mm(i=i, u=u, k=k, st=st):
                            # u1 accumulators borrow the (now idle) S-psum
                            # pool so each pool's ring is reused only every
                            # other strip -- the matmul then never WAR-waits
                            # on the previous strip's in-flight staging copy
                            if u == 0:
                                st["acc"] = mp.tile([128, 512], f32, tag="mm", name="mmy")
                            else:
                                st["acc"] = sp.tile([128, 2 * TQW], f32, tag="s", name="mmy")[:, 0:512]
                            nc.tensor.matmul(
                                st["acc"][:],
                                oc_s[1][:, 128 * i : 128 * (i + 1)],
                                wp_s[:, 1024 + 512 * u : 1024 + 512 * (u + 1)],
                                start=True,
                                stop=False,
                            )
                            nc.tensor.matmul(
                                st["acc"][:],
                                eye_s[:],
                                ys3_s[k][:, 512 * u : 512 * (u + 1)],
                                start=False,
                                stop=True,
                            )
                        units.append(mm)
                        def cp(u=u, st=st):
                            if u == 0:
                                nc.scalar.copy(st["yo"][:, 0:512], st["acc"][:])
                            else:
                                nc.vector.tensor_copy(st["yo"][:, 512:1024], st["acc"][:])
                        units.append(cp)
                    def out(i=i, st=st):
                        nc.sync.dma_start(y_d[128 * i : 128 * (i + 1), :], st["yo"][:])
                    units.append(out)
                return units

            def drain(units):
                for u in units:
                    u()
                units.clear()

            # ---- flat attention pipeline ----
            # One emitter for all (window, head-pair) segments. Emission is
            # software-pipelined so the PE queue never drains at a boundary:
            #   - each pair's O-matmuls are emitted one pair late (their exp
            #     has finished by the time they reach the queue head)
            #   - each segment's last-O flush + normalize is emitted after the
            #     NEXT segment's first S/exp, so ScalarE rolls straight on
            # Fillers live in one global queue with force-points for data
            # dependencies (a unit list must be fully emitted before the
            # instruction that reads its output is emitted).
            class FQ:
                def __init__(self):
                    self.q = []
                    self.added = 0
                    self.emitted = 0
                def add(self, units):
                    self.q += units
                    self.added += len(units)
                    return self.added  # mark
                def pop_some(self, n):
                    for _ in range(min(n, len(self.q))):
                        self.q.pop(0)()
                        self.emitted += 1
                def force_to(self, mark):
                    while self.emitted < mark and self.q:
                        self.q.pop(0)()
                        self.emitted += 1
                def drain(self):
                    self.pop_some(len(self.q))

            fm = FQ()
            marks = {}

            def attn_all():
                prev_fin = None
                nwin_blocks = [0]

                def pops(cap=6, dummy_ok=False):
                    avail = len(fm.q)
                    if avail > 0:
                        npop = min(cap, -(-avail // max(1, nwin_blocks[0])))
                        fm.pop_some(npop)

                for g in range(NGRP):
                    w0 = TQW * g
                    npairs = 2 * g + 2
                    nwin_blocks[0] = 2 * 2 * npairs
                    # window fillers (kq first: forced by next window's start)
                    if g == 0:
                        marks["v01"] = fm.add(v_units(0, js=(0, 1)))
                        marks["f13"] = fm.add(kq_units(0, fs=(1, 3)))
                        marks["v023"] = fm.add(v_units(0, js=(2, 3)))
                        # a window's first S-pairs read only its q features;
                        # its own k columns are first touched at pair m=2g,
                        # so the k units get their own (later) force point
                        marks["kq1q"] = fm.add(kq_units(1, fs=(2, 3)))
                        marks["kq1k"] = fm.add(kq_units(1, fs=(0, 1)))
                        marks["v1"] = fm.add(v_units(1))
                    elif g == 1:
                        marks["kq2q"] = fm.add(kq_units(2, fs=(2, 3)))
                        marks["kq2k"] = fm.add(kq_units(2, fs=(0, 1)))
                    elif g == 2:
                        # v2 spills from window 1 (only needed by pair m=4);
                        # proj1 moves to window 3, whose exp shadow has room
                        marks["v2"] = fm.add(v_units(2))
                        marks["kq3q"] = fm.add(kq_units(3, fs=(2, 3)))
                        marks["v3"] = fm.add(v_units(3))
                        fm.add(proj_units(0))
                    else:
                        marks["kq3k"] = fm.add(kq_units(3, fs=(0, 1)))
                        fm.add(proj_units(1) + proj_units(2))
                    if g > 0:
                        fm.force_to(marks[{1: "kq1q", 2: "kq2q", 3: "kq3q"}[g]])
                    for hp in range(2):
                        h0, h1 = 2 * hp, 2 * hp + 1
                        if g == 0 and hp == 1:
                            fm.force_to(marks["f13"])
                        o_t = {}
                        pending_o = None
                        for m in range(npairs):
                            lowp = not (g == 0 and m == 0)
                            if lowp:
                                pt2 = ptp.tile([128, 4 * TQW], f8, tag="pt", name="pt")
                            else:
                                pt2 = ptp.tile([128, 4 * TQW], bf16, tag="ptb", name="ptb")
                            cs0 = max(0, 128 * (2 * m) - w0)
                            if g > 0 and m == 2 * g:
                                fm.force_to(
                                    marks[{1: "kq1k", 2: "kq2k", 3: "kq3k"}[g]]
                                )
                            for p in range(2):
                                j = 2 * m + p
                                cs = max(0, 128 * j - w0)
                                s_t = sp.tile([128, 2 * TQW], f32, tag="s", name="s")
                                for idx, h in enumerate((h0, h1)):
                                    kT = kq_s[h // 2][64 * (h % 2) : 64 * (h % 2) + 64, :]
                                    qT = kq_s[2 + h // 2][64 * (h % 2) : 64 * (h % 2) + 64, :]
                                    nc.tensor.matmul(
                                        s_t[:, 512 * idx + cs : 512 * idx + 512],
                                        kT[:, 128 * j : 128 * (j + 1)],
                                        qT[:, w0 + cs : w0 + TQW],
                                        start=True,
                                        stop=True,
                                    )
                                po = 1024 * p
                                if cs:
                                    # diagonal: one 3D-AP activation covers both
                                    # heads' [cs:512] strips (saves the second
                                    # instruction's ~350-cycle overhead)
                                    nc.scalar.activation(
                                        pt2[:, po : po + 1024].rearrange(
                                            "p (two x) -> p two x", two=2
                                        )[:, :, cs:512],
                                        s_t[:].rearrange("p (two x) -> p two x", two=2)[
                                            :, :, cs:512
                                        ],
                                        mybir.ActivationFunctionType.Exp,
                                        scale=float(D) ** -0.5,
                                    )
                                else:
                                    nc.scalar.activation(
                                        pt2[:, po : po + 2 * TQW],
                                        s_t[:, 0 : 2 * TQW],
                                        mybir.ActivationFunctionType.Exp,
                                        scale=float(D) ** -0.5,
                                    )
                                # previous segment's O-flush + normalize must be
                                # emitted before any filler pops here: spilled
                                # proj/proj3a units read the oc it writes
                                if m == 0 and p == 0 and prev_fin is not None:
                                    prev_fin()
                                    prev_fin = None
                                # diagonal blocks have little exp shadow --
                                # defer filler to the next window's long exps
                                # (not in window 3: its spill would hit the tail)
                                pops(cap=8 if g == 0 else (2 if (cs and g < 3) else 6))
                                nwin_blocks[0] -= 1
                                if 128 * j >= w0:
                                    if p == 1 and cs > cs0:
                                        for idx in range(2):
                                            nc.gpsimd.memset(
                                                pt2[:, po + 512 * idx + cs0 : po + 512 * idx + cs],
                                                0.0,
                                            )
                                    for idx in range(2):
                                        nc.gpsimd.affine_select(
                                            out=pt2[:, po + 512 * idx + cs : po + 512 * idx + cs + 128],
                                            in_=pt2[:, po + 512 * idx + cs : po + 512 * idx + cs + 128],
                                            compare_op=mybir.AluOpType.is_ge,
                                            fill=0.0,
                                            base=0,
                                            pattern=[[1, 128]],
                                            channel_multiplier=-1,
                                        )
                            if pending_o is not None:
                                pending_o()
                            def o_emit(m=m, pt2=pt2, cs0=cs0, lowp=lowp, o_t=o_t,
                                       h0=h0, h1=h1, npairs=npairs, w0=w0, g=g):
                                if not o_t:
                                    o_t[h0] = op.tile([128, TQW], f32, tag="oh0", name="oh0")
                                    o_t[h1] = op.tile([128, TQW], f32, tag="oh1", name="oh1")
                                # v chunks 2m, 2m+1 must be resident
                                if g == 0:
                                    fm.force_to(marks["v01"] if m == 0 else marks["v023"])
                                elif m == 2 * g:
                                    fm.force_to(marks[f"v{g}"])
                                if lowp:
                                    for idx, h in enumerate((h0, h1)):
                                        vv = v_s[:].rearrange("p (jj x) -> p jj x", jj=16)[
                                            :, 2 * m : 2 * m + 2, 128 * h : 128 * (h + 1)
                                        ]
                                        rr = pt2[:].rearrange("p (jj x) -> p jj x", jj=2)[
                                            :, :, 512 * idx + cs0 : 512 * (idx + 1)
                                        ]
                                        nc.tensor.matmul(
                                            o_t[h][:, cs0:TQW],
                                            vv,
                                            rr,
                                            start=(m == 0),
                                            stop=(m == npairs - 1),
                                            perf_mode=mybir.MatmulPerfMode.DoubleRow,
                                        )
                                else:
                                    for p_ in range(2):
                                        jj = 2 * m + p_
                                        jcs = max(0, 128 * jj - w0)
                                        for idx, h in enumerate((h0, h1)):
                                            nc.tensor.matmul(
                                                o_t[h][:, jcs:TQW],
                                                vb_s[:, 512 * p_ + 128 * h : 512 * p_ + 128 * (h + 1)],
                                                pt2[:, 1024 * p_ + 512 * idx + jcs : 1024 * p_ + 512 * (idx + 1)],
                                                start=(jj == 0),
                                                stop=False,
                                            )
                            pending_o = o_emit
                        pops(cap=2)

                        def fin(pending_o=pending_o, o_t=o_t, h0=h0, h1=h1, w0=w0,
                                last=(g == NGRP - 1 and hp == 1)):
                            pending_o()
                            # both heads' denominators stacked in one tile so
                            # a single reciprocal instruction covers them
                            lsb = rp.tile([128, 512], f32, tag="lsb", name="lsb")
                            for idx, h in enumerate((h0, h1)):
                                if last:
                                    # tail: ScalarE is free, keep DVE for the
                                    # recip/mult/proj3b chain
                                    nc.scalar.copy(lsb[64 * idx : 64 * idx + 64, :], o_t[h][64:128, :])
                                else:
                                    nc.vector.tensor_copy(lsb[64 * idx : 64 * idx + 64, :], o_t[h][64:128, :])
                            if last:
                                # keep-warm: cast a strip of lsb into the wu
                                # tile (ScalarE, gated on the copies), then a
                                # full-rate bf16 matmul runs mid-normalize so
                                # HAM stays at 8/8 for the tail matmuls
                                nc.scalar.copy(wu_s[0:64, 0:128], lsb[64:128, 0:128])
                                wacc = mp.tile([128, 512], f32, tag="mm", name="warm")
                                nc.tensor.matmul(
                                    wacc[:], wu_s[:, 0:128], wu_s[:],
                                    start=True, stop=True,
                                )
                            rinv_t = rp.tile([128, 512], f32, tag="rinv", name="rinv")
                            nc.vector.reciprocal_approx_fast(rinv_t[:], lsb[:])
                            rinv = {h0: rinv_t[0:64, :], h1: rinv_t[64:128, :]}

                            for h in (h0, h1):
                                nc.vector.tensor_tensor(
                                    oc_s[h // 2][
                                        64 * (h % 2) : 64 * (h % 2) + 64, w0 : w0 + TQW
                                    ],
                                    o_t[h][0:64, :],
                                    rinv[h],
                                    mybir.AluOpType.mult,
                                )
                        prev_fin = fin
                        if g == 3 and hp == 0:
                            fm.add(proj3_a_units())
                # final segment normalize (with keep-warm matmuls inside)
                prev_fin()
                fm.drain()

            # ---- schedule ----
            drain(kq_units(0, fs=(0, 2)))
            attn_all()
            drain(proj3_b_units())

    nc.compile()
    return nc


def get_nc():
    if "nc" not in _nc_cache:
        _nc_cache["nc"] = _build_bass()
    return _nc_cache["nc"]


def _shard_inputs(x, W_kqv, W_proj):
    """Build the 8 per-core input maps: shard, transpose, cast to bf16 and
    pack 128-row panels along the free dim."""
    bf16 = ml_dtypes.bfloat16

    def pack(a):  # [128*k, n] -> [128, k*n], panel-major along free dim
        k = a.shape[0] // 128
        return np.ascontiguousarray(
            a.reshape(k, 128, a.shape[1]).transpose(1, 0, 2).reshape(128, -1)
        ).astype(bf16)

    f8 = ml_dtypes.float8_e4m3

    in_maps = []
    for core in range(8):
        b, hg = core // 4, core % 4
        heads = range(4 * hg, 4 * hg + 4)
        xt = x[b].T  # [C, T]
        # bf16 xt, window 0 only: [128, c*512 + t']
        xtw = xt.reshape(8, 128, 4, 512)  # [c, p, g, t']
        xtp = np.ascontiguousarray(xtw[:, :, 0].transpose(1, 0, 2).reshape(128, -1)).astype(bf16)
        # fp8 xt, windows 1-3, contraction pairs: [128, (g-1)*4096 + cp*1024 + par*512 + t']
        xtf = np.ascontiguousarray(
            xt.reshape(4, 2, 128, 4, 512)[:, :, :, 1:]  # [cp, par, p, g-1, t']
            .transpose(2, 3, 0, 1, 4)
            .reshape(128, -1)
        ).astype(f8)
        k_rows = [W_kqv[64 * h : 64 * (h + 1)] for h in heads]
        q_rows = [W_kqv[C + 64 * h : C + 64 * (h + 1)] for h in heads]
        v_rows = [W_kqv[2 * C + 64 * h : 2 * C + 64 * (h + 1)] for h in heads]
        # feature-tile order [k01, q01, k23, q23] (see FBLK in _build_bass)
        wqk_cat = np.concatenate(
            k_rows[0:2] + q_rows[0:2] + k_rows[2:4] + q_rows[2:4], 0
        )  # [512 feat, 1024 c]
        # f-major packing: [p, f*1024 + c*128 + fi]
        wqk = np.ascontiguousarray(
            wqk_cat.reshape(4, 128, 8, 128).transpose(3, 0, 2, 1).reshape(128, -1)
        ).astype(bf16)
        # fp8 SwInterleave packing: per (f, cp) block of 256, the physical
        # column sequence is [A_f127, B_f127, ..., A_f0, B_f0] where A/B are
        # the two contraction-pair halves (par) and features run reversed --
        # the layout the PE's DoubleRowSwInterleave weight load expects
        wqkf = np.ascontiguousarray(
            wqk_cat.reshape(4, 128, 4, 2, 128)[:, ::-1]  # [f, fi_rev, cp, par, ci]
            .transpose(4, 0, 2, 1, 3)                    # [ci, f, cp, fi_rev, par]
            .reshape(128, -1)
        ).astype(f8)
        wv_cat = np.concatenate(v_rows, 0).T  # [1024 c, 256]
        wv = pack(wv_cat)
        # fp8 pair packing: [p, cp*512 + par*256 + x]
        wvf = np.ascontiguousarray(
            wv_cat.reshape(4, 2, 128, 256).transpose(2, 0, 1, 3).reshape(128, -1)
        ).astype(f8)
        wp = pack(W_proj[:, 256 * hg : 256 * (hg + 1)].T)
        in_maps.append(
            {"xt": xtp, "xtf": xtf, "wqk": wqk, "wqkf": wqkf,
             "wv": wv, "wvf": wvf, "wp": wp,
             "eye": np.eye(128, dtype=bf16)}
        )
    return in_maps


def kernel(x, W_kqv, W_proj, b_proj):
    from concourse.bass_utils import run_bass_kernel_spmd

    x = np.asarray(x, dtype=np.float32)
    W_kqv = np.asarray(W_kqv, dtype=np.float32)
    W_proj = np.asarray(W_proj, dtype=np.float32)
    b_proj = np.asarray(b_proj, dtype=np.float32)
    nc = get_nc()
    in_maps = _shard_inputs(x, W_kqv, W_proj)
    res = run_bass_kernel_spmd(nc, in_maps, core_ids=list(range(8)))
    B = x.shape[0]
    out = np.empty((B, T, C), np.float32)
    for b in range(B):
        acc = res.results[4 * b]["y"].astype(np.float32).copy()
        for hg in range(1, 4):
            acc += res.results[4 * b + hg]["y"]
        out[b] = acc + b_proj[None, :]
    return out



# revision 55
# speedup vs baseline: 1.2007x; 1.2007x over previous
"""Causal self-attention kernel for 8 trn2 NeuronCores.

Sharding: core c handles batch b = c // 4 and local head group hg = c % 4
(4 of the 16 heads). Tensor-parallel over heads for kqv / attention and
row-parallel for the output projection; the 4 per-batch partial projections
(bf16) are summed on the host, where the bias is also added.

Inputs are pre-tiled on the host into [128, *] SBUF-ready layouts so every
load is a large multi-queue DMA. Feature tiles are packed [k01,q01,k23,q23]
so the first DMA chunk covers exactly what the first attention blocks need.

Device kernel (per core): one flat software-pipelined emitter over all
(window, head-pair) segments:
  - each pair's O-matmuls are emitted one pair late, and each segment's
    O-flush + normalize is emitted after the NEXT segment's first S/exp, so
    neither the PE queue nor ScalarE drains at hp/window boundaries (a
    drained PE queue also defeats the background LDWEIGHTS prefetch, which
    is what lets back-to-back matmuls sustain ~216ns/512cols)
  - dense work (later windows' k/q/v projections, earlier windows' output
    projections) lives in one global filler queue, popped between exp
    emissions proportionally to the remaining blocks; force-points drain a
    unit group before the instruction that reads its output is emitted
  - fp8e4: kq for windows 1-3 uses DoubleRowSwInterleave with host-packed
    interleaved+reversed weights (contiguous fast weight load, full 2x fp8
    column rate); v projections and attention O-matmuls use DoubleRow pair
    views; window 0 kq/v and the output projection stay bf16 (fp8 noise on
    short-context rows / in the projection reaches the output unaveraged
    and blows the 2e-2 budget - measured, not theoretical)
  - exp strips: S^T psum -> ScalarE exp -> fp8 [j0:h0|h1 | j1:h0|h1] tile;
    diagonal blocks use one 3D-AP activation covering both heads' [cs:512]
    strips (saves the 2nd instruction's ~350-cycle overhead); causal masks
    via GpSimd affine_select; denominators via ones-columns in the O
    stationary (psum rows 64:128 accumulate sum(P) for free)
  - tail: window 3's oc_s[0] projection half (proj3_a) runs as filler in
    the hp=1 exp shadow; after the last normalize (ScalarE lsb copies + a
    gated bf16 keep-warm matmul), proj3_b folds the bf16 a-half into psum
    with an identity matmul and stages out via ScalarE/DVE copies, each
    128-row y strip DMA'd (bf16) as soon as it completes

Per tq window g of 512 (pairs m <= 2g+1, causally trimmed):
  S^T = k^T.T q^T   (K=64; the head pair runs concurrently on PE row
                     groups 0-63/64-127 via base_partition tile_position)
  P = exp(S^T/8)    no max subtraction (scores are O(1))
  O^T psum[0:64] += [v_2m v_2m+1] @ [P_2m P_2m+1]   (fp8 DoubleRow)
  normalize: copies stack both heads' denominators in one tile, one
             reciprocal_approx_fast covers both, then multiply -> oc bf16
  proj: y[:, window] = O_cat^T.T @ Wp^T -> bf16 -> DMA
"""

import numpy as np
import ml_dtypes

T = 2048
C = 1024
NH_LOCAL = 4
D = 64
TQW = 512  # tq window width
NGRP = T // TQW  # 4 tq windows

_nc_cache = {}


def _build_bass():
    import concourse.mybir as mybir
    import concourse.tile as tile
    from concourse import bacc

    f32 = mybir.dt.float32
    bf16 = mybir.dt.bfloat16
    f8 = mybir.dt.float8e4

    nc = bacc.Bacc(None, target_bir_lowering=False)
    # pre-tiled inputs: [128, packed free dim] (see _shard_inputs)
    # bf16 copies feed window 0 (short-context rows need the precision);
    # fp8 pair-interleaved copies feed windows 1-3 via DoubleRow matmuls
    xt_d = nc.dram_tensor("xt", [128, 4096], bf16, kind="ExternalInput")
    xtf_d = nc.dram_tensor("xtf", [128, 3 * 4096], f8, kind="ExternalInput")
    wqk_d = nc.dram_tensor("wqk", [128, 8 * 512], bf16, kind="ExternalInput")
    wqkf_d = nc.dram_tensor("wqkf", [128, 4096], f8, kind="ExternalInput")
    wv_d = nc.dram_tensor("wv", [128, 8 * 256], bf16, kind="ExternalInput")
    wvf_d = nc.dram_tensor("wvf", [128, 2048], f8, kind="ExternalInput")
    wp_d = nc.dram_tensor("wp", [128, 2 * C], bf16, kind="ExternalInput")
    eye_d = nc.dram_tensor("eye", [128, 128], bf16, kind="ExternalInput")
    y_d = nc.dram_tensor("y", [T, C], bf16, kind="ExternalOutput")

    # host packs wqk/wqkf feature tiles in order [k01, q01, k23, q23] so the
    # first DMA chunk covers everything attn(0) hp=0 needs; f is the semantic
    # tile index (0=k01, 1=k23, 2=q01, 3=q23) used by kq_s / attn
    FBLK = {0: 0, 2: 1, 1: 2, 3: 3}

    with tile.TileContext(nc) as tc:
        with (
            tc.tile_pool(name="persist", bufs=1) as pp,
            tc.tile_pool(name="mmp", bufs=2, space="PSUM") as mp,
            tc.tile_pool(name="spsum", bufs=2, space="PSUM") as sp,
            tc.tile_pool(name="opsum", bufs=1, space="PSUM") as op,
            tc.tile_pool(name="ptp", bufs=4) as ptp,
            tc.tile_pool(name="rp", bufs=4) as rp,
            tc.tile_pool(name="ysb", bufs=3) as ysb,
        ):
            xt_a = pp.tile([128, 2048], bf16, tag="xta", name="xta")
            xt_b = pp.tile([128, 2048], bf16, tag="xtb", name="xtb")
            xtf_s = pp.tile([128, 3 * 4096], f8, tag="xtf", name="xtf")
            wqk_a = pp.tile([128, 2048], bf16, tag="wqka", name="wqka")
            wqk_b = pp.tile([128, 2048], bf16, tag="wqkb", name="wqkb")
            wqkf_s = pp.tile([128, 4096], f8, tag="wqkf", name="wqkf")
            wv_s = pp.tile([128, 8 * 256], bf16, tag="wv", name="wv")
            wvf_s = pp.tile([128, 2048], f8, tag="wvf", name="wvf")
            wp_s = pp.tile([128, 2 * C], bf16, tag="wp", name="wp")
            eye_s = pp.tile([128, 128], bf16, tag="eye", name="eye")
            kq_s = [pp.tile([128, T], bf16, tag=f"kq{f}", name=f"kq{f}") for f in range(4)]
            v_s = pp.tile([128, 4 * T], f8, tag="vall", name="vall")
            oc_s = [pp.tile([128, T], bf16, tag=f"oc{p}", name=f"oc{p}") for p in range(2)]
            wu_s = pp.tile([128, 512], bf16, tag="wu", name="wu")
            # bf16 copy of tk chunks j=0,1 (v | ones): short-context queries
            # (t<256) see little averaging, too noisy for the fp8 path
            vb_s = pp.tile([128, 1024], bf16, tag="vb", name="vb")
            # window-3 proj strips held in SBUF across the hp=0/hp=1 split
            ys3_s = [
                pp.tile([128, C], bf16, tag=f"ys3_{i}", name=f"ys3_{i}")
                for i in range(4)
            ]

            # slicing helpers for the packed layouts
            def xt_w(c):  # bf16 moving operand, window 0, contraction chunk c
                t = xt_a if c < 4 else xt_b
                return t[:, 512 * (c % 4) : 512 * (c % 4 + 1)]

            def xt_j(c, j):  # bf16 stationary for v chunks 0-3
                t = xt_a if c < 4 else xt_b
                o = 512 * (c % 4) + 128 * (j % 4)
                return t[:, o : o + 128]

            def wqk_blk(b):  # bf16 feature block b (packed order)
                t = wqk_a if b < 2 else wqk_b
                return t[:, 1024 * (b % 2) : 1024 * (b % 2 + 1)]

            def xtf_w(cp, g):  # fp8 [p,2,512] rhs pair view, windows 1-3
                o = 4096 * (g - 1) + 1024 * cp
                return xtf_s[:, o : o + 1024].rearrange(
                    "p (two x) -> p two x", two=2
                )

            def xtf_j(cp, j):  # fp8 [p,2,128] stationary pair view, chunks 4-15
                o = 4096 * (j // 4 - 1) + 1024 * cp
                t0 = 128 * (j % 4)
                return xtf_s[:, o : o + 1024].rearrange(
                    "p (two x) -> p two x", two=2
                )[:, :, t0 : t0 + 128]

            # ---- HAM warmup: PE busy from boot so real matmuls run warm ----
            # short: just enough to cover the first DMA chunk's latency; the
            # first kq chains continue the warm streak
            nc.gpsimd.memset(wu_s[:], 0.03125)
            for _ in range(12):
                wacc = mp.tile([128, 512], f32, tag="mm", name="warm")
                nc.tensor.matmul(wacc[:], wu_s[:, 0:128], wu_s[:], start=True, stop=True)

            # ones blocks for the denominator trick (v regions overwritten
            # later); contiguous memset on GpSimd - off Vector's critical path
            nc.gpsimd.memset(v_s[:], 1.0)
            nc.gpsimd.memset(vb_s[:], 1.0)

            # input DMAs: few large issues ordered by first use (a single
            # dma_start is split across all 16 queues by the framework)
            nc.sync.dma_start(wqk_a[:], wqk_d[:, 0:2048])
            nc.sync.dma_start(xt_a[:], xt_d[:, 0:2048])
            nc.sync.dma_start(xt_b[:], xt_d[:, 2048:4096])
            nc.sync.dma_start(wv_s[:], wv_d[:])
            nc.sync.dma_start(wqk_b[:], wqk_d[:, 2048:4096])
            nc.sync.dma_start(wqkf_s[:], wqkf_d[:])
            nc.sync.dma_start(xtf_s[:, 0:4096], xtf_d[:, 0:4096])
            nc.sync.dma_start(wvf_s[:], wvf_d[:])
            nc.sync.dma_start(wp_s[:], wp_d[:])
            nc.sync.dma_start(xtf_s[:, 4096:12288], xtf_d[:, 4096:12288])
            nc.sync.dma_start(eye_s[:], eye_d[:])

            # ---- dense work generators (emitted one instruction at a time) ----
            def kq_units(g, fs=(0, 1, 2, 3)):
                units = []
                for f in fs:
                    b = FBLK[f]
                    st = {}
                    if g == 0:
                        for c in range(8):
                            def mm(b=b, c=c, st=st):
                                if c == 0:
                                    st["acc"] = mp.tile([128, 512], f32, tag="mm", name="mmkq")
                                nc.tensor.matmul(
                                    st["acc"][:],
                                    wqk_blk(b)[:, 128 * c : 128 * (c + 1)],
                                    xt_w(c),
                                    start=(c == 0),
                                    stop=(c == 7),
                                )
                            units.append(mm)
                    else:
                        for cp_ in range(4):
                            def mm(b=b, cp_=cp_, g=g, st=st):
                                if cp_ == 0:
                                    st["acc"] = mp.tile([128, 512], f32, tag="mm", name="mmkq")
                                # SwInterleave: wqkf is host-packed with the
                                # A/B contraction pair interleaved per feature
                                # column (reversed), so LDWEIGHTS is one
                                # contiguous fast load and the fp8 matmul
                                # sustains its full 2x column rate
                                nc.tensor.matmul(
                                    st["acc"][:],
                                    wqkf_s[
                                        :, 1024 * b + 256 * cp_ : 1024 * b + 256 * (cp_ + 1)
                                    ],
                                    xtf_w(cp_, g),
                                    start=(cp_ == 0),
                                    stop=(cp_ == 3),
                                    perf_mode=mybir.MatmulPerfMode.DoubleRowSwInterleave,
                                )
                            units.append(mm)
                    def cp(f=f, g=g, st=st):
                        nc.vector.tensor_copy(
                            kq_s[f][:, TQW * g : TQW * (g + 1)], st["acc"][:]
                        )
                    units.append(cp)
                return units

            def v_units(g, js=None):
                units = []
                for j in js if js is not None else range(4 * g, 4 * g + 4):
                    st = {}
                    if g == 0:
                        for c in range(8):
                            def mm(j=j, c=c, st=st):
                                if c == 0:
                                    st["acc"] = mp.tile([128, 512], f32, tag="mm", name="mmv")
                                nc.tensor.matmul(
                                    st["acc"][:, :256],
                                    xt_j(c, j),
                                    wv_s[:, 256 * c : 256 * (c + 1)],
                                    start=(c == 0),
                                    stop=(c == 7),
                                )
                            units.append(mm)
                    else:
                        for cp_ in range(4):
                            def mm(j=j, cp_=cp_, st=st):
                                if cp_ == 0:
                                    st["acc"] = mp.tile([128, 512], f32, tag="mm", name="mmv")
                                nc.tensor.matmul(
                                    st["acc"][:, :256],
                                    xtf_j(cp_, j),
                                    wvf_s[:, 512 * cp_ : 512 * (cp_ + 1)].rearrange(
                                        "p (two x) -> p two x", two=2
                                    ),
                                    start=(cp_ == 0),
                                    stop=(cp_ == 3),
                                    perf_mode=mybir.MatmulPerfMode.DoubleRow,
                                )
                            units.append(mm)
                    def cp(j=j, st=st):
                        nc.vector.tensor_copy(
                            v_s[:, 512 * j : 512 * j + 512].rearrange(
                                "p (h x) -> p h x", h=4
                            )[:, :, 0:64],
                            st["acc"][:, 0:256].rearrange("p (h x) -> p h x", h=4),
                        )
                        if j < 2:
                            nc.vector.tensor_copy(
                                vb_s[:, 512 * j : 512 * (j + 1)].rearrange(
                                    "p (h x) -> p h x", h=4
                                )[:, :, 0:64],
                                st["acc"][:, 0:256].rearrange("p (h x) -> p h x", h=4),
                            )
                    units.append(cp)
                return units

            def proj_units(g):
                # windows 0..2: full projection, y strips DMA'd in pairs
                units = []
                for pi in range(2):
                    i0 = 4 * g + 2 * pi
                    st = {}
                    def alloc(st=st):
                        st["ys"] = ysb.tile([128, 2 * C], bf16, tag="ys", name="ys")
                    units.append(alloc)
                    for k in range(2):
                        i = i0 + k
                        for u in range(2):
                            for ci in range(2):
                                def mm(i=i, u=u, ci=ci, st=st):
                                    if ci == 0:
                                        st["acc"] = mp.tile([128, 512], f32, tag="mm", name="mmy")
                                    nc.tensor.matmul(
                                        st["acc"][:],
                                        oc_s[ci][:, 128 * i : 128 * (i + 1)],
                                        wp_s[:, 1024 * ci + 512 * u : 1024 * ci + 512 * (u + 1)],
                                        start=(ci == 0),
                                        stop=(ci == 1),
                                    )
                                units.append(mm)
                            def cp(k=k, u=u, st=st):
                                nc.vector.tensor_copy(
                                    st["ys"][:, 1024 * k + 512 * u : 1024 * k + 512 * (u + 1)],
                                    st["acc"][:],
                                )
                            units.append(cp)
                    def out(i0=i0, st=st):
                        nc.sync.dma_start(
                            y_d[128 * i0 : 128 * i0 + 256, :].rearrange(
                                "(two p) c -> p two c", two=2
                            ),
                            st["ys"][:].rearrange("p (two c) -> p two c", two=2),
                        )
                    units.append(out)
                return units

            def proj3_a_units():
                # window 3, oc_s[0] (head pair 0) half: filler for attn(3)
                # hp=1; partial y strips stream out early (bypass DMA)
                units = []
                for k, i in enumerate(range(12, 16)):
                    st = {}
                    for u in range(2):
                        def mm(i=i, u=u, st=st):
                            st["acc"] = mp.tile([128, 512], f32, tag="mm", name="mmy")
                            nc.tensor.matmul(
                                st["acc"][:],
                                oc_s[0][:, 128 * i : 128 * (i + 1)],
                                wp_s[:, 512 * u : 512 * (u + 1)],
                                start=True,
                                stop=True,
                            )
                        units.append(mm)
                        def cp(k=k, u=u, st=st):
                            nc.vector.tensor_copy(
                                ys3_s[k][:, 512 * u : 512 * (u + 1)], st["acc"][:]
                            )
                        units.append(cp)
                return units

            def proj3_b_units():
                # window 3, oc_s[1] half: the bf16 a-half is folded into the
                # psum with an identity matmul (PE), then the psum is staged
                # out with one ScalarE copy and one DVE copy per strip so no
                # single engine serializes the tail
                units = []
                for k, i in enumerate(range(12, 16)):
                    st = {}
                    def alloc(st=st):
                        st["yo"] = ysb.tile([128, C], bf16, tag="yo", name="yo")
                    units.append(alloc)
                    for u in range(2):
                        def mm(i=i, u=u, k=k, st=st):
                            # u1 accumulators borrow the (now idle) S-psum
                            # pool so each pool's ring is reused only every
                            # other strip -- the matmul then never WAR-waits
                            # on the previous strip's in-flight staging copy
                            if u == 0:
                                st["acc"] = mp.tile([128, 512], f32, tag="mm", name="mmy")
                            else:
                                st["acc"] = sp.tile([128, 2 * TQW], f32, tag="s", name="mmy")[:, 0:512]
                            nc.tensor.matmul(
                                st["acc"][:],
                                oc_s[1][:, 128 * i : 128 * (i + 1)],
                                wp_s[:, 1024 + 512 * u : 1024 + 512 * (u + 1)],
                                start=True,
                                stop=False,
                            )
                            nc.tensor.matmul(
                                st["acc"][:],
                                eye_s[:],
                                ys3_s[k][:, 512 * u : 512 * (u + 1)],
                                start=False,
                                stop=True,
                            )
                        units.append(mm)
                        def cp(u=u, st=st):
                            if u == 0:
                                nc.scalar.copy(st["yo"][:, 0:512], st["acc"][:])
                            else:
                                nc.vector.tensor_copy(st["yo"][:, 512:1024], st["acc"][:])
                        units.append(cp)
                    def out(i=i, st=st):
                        nc.sync.dma_start(y_d[128 * i : 128 * (i + 1), :], st["yo"][:])
                    units.append(out)
                return units

            def drain(units):
                for u in units:
                    u()
                units.clear()

            # ---- flat attention pipeline ----
            # One emitter for all (window, head-pair) segments. Emission is
            # software-pipelined so the PE queue never drains at a boundary:
            #   - each pair's O-matmuls are emitted one pair late (their exp
            #     has finished by the time they reach the queue head)
            #   - each segment's last-O flush + normalize is emitted after the
            #     NEXT segment's first S/exp, so ScalarE rolls straight on
            # Fillers live in one global queue with force-points for data
            # dependencies (a unit list must be fully emitted before the
            # instruction that reads its output is emitted).
            class FQ:
                def __init__(self):
                    self.q = []
                    self.added = 0
                    self.emitted = 0
                def add(self, units):
                    self.q += units
                    self.added += len(units)
                    return self.added  # mark
                def pop_some(self, n):
                    for _ in range(min(n, len(self.q))):
                        self.q.pop(0)()
                        self.emitted += 1
                def force_to(self, mark):
                    while self.emitted < mark and self.q:
                        self.q.pop(0)()
                        self.emitted += 1
                def drain(self):
                    self.pop_some(len(self.q))

            fm = FQ()
            marks = {}

            def attn_all():
                prev_fin = None
                nwin_blocks = [0]

                def pops(cap=6, dummy_ok=False):
                    avail = len(fm.q)
                    if avail > 0:
                        npop = min(cap, -(-avail // max(1, nwin_blocks[0])))
                        fm.pop_some(npop)

                for g in range(NGRP):
                    w0 = TQW * g
                    npairs = 2 * g + 2
                    nwin_blocks[0] = 2 * 2 * npairs
                    # window fillers (kq first: forced by next window's start)
                    if g == 0:
                        marks["v01"] = fm.add(v_units(0, js=(0, 1)))
                        marks["f13"] = fm.add(kq_units(0, fs=(1, 3)))
                        marks["v023"] = fm.add(v_units(0, js=(2, 3)))
                        marks["kq1"] = fm.add(kq_units(1))
                        marks["v1"] = fm.add(v_units(1))
                    elif g == 1:
                        marks["kq2"] = fm.add(kq_units(2))
                    elif g == 2:
                        # v2 spills from window 1 (only needed by pair m=4);
                        # proj1 moves to window 3, whose exp shadow has room
                        marks["v2"] = fm.add(v_units(2))
                        marks["kq3q"] = fm.add(kq_units(3, fs=(2, 3)))
                        marks["v3"] = fm.add(v_units(3))
                        fm.add(proj_units(0))
                    else:
                        marks["kq3k"] = fm.add(kq_units(3, fs=(0, 1)))
                        fm.add(proj_units(1) + proj_units(2))
                    if g > 0:
                        fm.force_to(marks[f"kq{g}"] if g < 3 else marks["kq3q"])
                    for hp in range(2):
                        h0, h1 = 2 * hp, 2 * hp + 1
                        if g == 0 and hp == 1:
                            fm.force_to(marks["f13"])
                        o_t = {}
                        pending_o = None
                        for m in range(npairs):
                            lowp = not (g == 0 and m == 0)
                            if lowp:
                                pt2 = ptp.tile([128, 4 * TQW], f8, tag="pt", name="pt")
                            else:
                                pt2 = ptp.tile([128, 4 * TQW], bf16, tag="ptb", name="ptb")
                            cs0 = max(0, 128 * (2 * m) - w0)
                            if g == 3 and m == 6:
                                fm.force_to(marks["kq3k"])
                            for p in range(2):
                                j = 2 * m + p
                                cs = max(0, 128 * j - w0)
                                s_t = sp.tile([128, 2 * TQW], f32, tag="s", name="s")
                                for idx, h in enumerate((h0, h1)):
                                    kT = kq_s[h // 2][64 * (h % 2) : 64 * (h % 2) + 64, :]
                                    qT = kq_s[2 + h // 2][64 * (h % 2) : 64 * (h % 2) + 64, :]
                                    nc.tensor.matmul(
                                        s_t[:, 512 * idx + cs : 512 * idx + 512],
                                        kT[:, 128 * j : 128 * (j + 1)],
                                        qT[:, w0 + cs : w0 + TQW],
                                        start=True,
                                        stop=True,
                                    )
                                po = 1024 * p
                                if cs:
                                    # diagonal: one 3D-AP activation covers both
                                    # heads' [cs:512] strips (saves the second
                                    # instruction's ~350-cycle overhead)
                                    nc.scalar.activation(
                                        pt2[:, po : po + 1024].rearrange(
                                            "p (two x) -> p two x", two=2
                                        )[:, :, cs:512],
                                        s_t[:].rearrange("p (two x) -> p two x", two=2)[
                                            :, :, cs:512
                                        ],
                                        mybir.ActivationFunctionType.Exp,
                                        scale=float(D) ** -0.5,
                                    )
                                else:
                                    nc.scalar.activation(
                                        pt2[:, po : po + 2 * TQW],
                                        s_t[:, 0 : 2 * TQW],
                                        mybir.ActivationFunctionType.Exp,
                                        scale=float(D) ** -0.5,
                                    )
                                # previous segment's O-flush + normalize must be
                                # emitted before any filler pops here: spilled
                                # proj/proj3a units read the oc it writes
                                if m == 0 and p == 0 and prev_fin is not None:
                                    prev_fin()
                                    prev_fin = None
                                # diagonal blocks have little exp shadow --
                                # defer filler to the next window's long exps
                                # (not in window 3: its spill would hit the tail)
                                pops(cap=8 if g == 0 else (2 if (cs and g < 3) else 6))
                                nwin_blocks[0] -= 1
                                if 128 * j >= w0:
                                    if p == 1 and cs > cs0:
                                        for idx in range(2):
                                            nc.gpsimd.memset(
                                                pt2[:, po + 512 * idx + cs0 : po + 512 * idx + cs],
                                                0.0,
                                            )
                                    for idx in range(2):
                                        nc.gpsimd.affine_select(
                                            out=pt2[:, po + 512 * idx + cs : po + 512 * idx + cs + 128],
                                            in_=pt2[:, po + 512 * idx + cs : po + 512 * idx + cs + 128],
                                            compare_op=mybir.AluOpType.is_ge,
                                            fill=0.0,
                                            base=0,
                                            pattern=[[1, 128]],
                                            channel_multiplier=-1,
                                        )
                            if pending_o is not None:
                                pending_o()
                            def o_emit(m=m, pt2=pt2, cs0=cs0, lowp=lowp, o_t=o_t,
                                       h0=h0, h1=h1, npairs=npairs, w0=w0, g=g):
                                if not o_t:
                                    o_t[h0] = op.tile([128, TQW], f32, tag="oh0", name="oh0")
                                    o_t[h1] = op.tile([128, TQW], f32, tag="oh1", name="oh1")
                                # v chunks 2m, 2m+1 must be resident
                                if g == 0:
                                    fm.force_to(marks["v01"] if m == 0 else marks["v023"])
                                elif m == 2 * g:
                                    fm.force_to(marks[f"v{g}"])
                                if lowp:
                                    for idx, h in enumerate((h0, h1)):
                                        vv = v_s[:].rearrange("p (jj x) -> p jj x", jj=16)[
                                            :, 2 * m : 2 * m + 2, 128 * h : 128 * (h + 1)
                                        ]
                                        rr = pt2[:].rearrange("p (jj x) -> p jj x", jj=2)[
                                            :, :, 512 * idx + cs0 : 512 * (idx + 1)
                                        ]
                                        nc.tensor.matmul(
                                            o_t[h][:, cs0:TQW],
                                            vv,
                                            rr,
                                            start=(m == 0),
                                            stop=(m == npairs - 1),
                                            perf_mode=mybir.MatmulPerfMode.DoubleRow,
                                        )
                                else:
                                    for p_ in range(2):
                                        jj = 2 * m + p_
                                        jcs = max(0, 128 * jj - w0)
                                        for idx, h in enumerate((h0, h1)):
                                            nc.tensor.matmul(
                                                o_t[h][:, jcs:TQW],
                                                vb_s[:, 512 * p_ + 128 * h : 512 * p_ + 128 * (h + 1)],
                                                pt2[:, 1024 * p_ + 512 * idx + jcs : 1024 * p_ + 512 * (idx + 1)],
                                                start=(jj == 0),
                                                stop=False,
                                            )
                            pending_o = o_emit
                        pops(cap=2)

                        def fin(pending_o=pending_o, o_t=o_t, h0=h0, h1=h1, w0=w0,
                                last=(g == NGRP - 1 and hp == 1)):
                            pending_o()
                            # both heads' denominators stacked in one tile so
                            # a single reciprocal instruction covers them
                            lsb = rp.tile([128, 512], f32, tag="lsb", name="lsb")
                            for idx, h in enumerate((h0, h1)):
                                if last:
                                    # tail: ScalarE is free, keep DVE for the
                                    # recip/mult/proj3b chain
                                    nc.scalar.copy(lsb[64 * idx : 64 * idx + 64, :], o_t[h][64:128, :])
                                else:
                                    nc.vector.tensor_copy(lsb[64 * idx : 64 * idx + 64, :], o_t[h][64:128, :])
                            if last:
                                # keep-warm: cast a strip of lsb into the wu
                                # tile (ScalarE, gated on the copies), then a
                                # full-rate bf16 matmul runs mid-normalize so
                                # HAM stays at 8/8 for the tail matmuls
                                nc.scalar.copy(wu_s[0:64, 0:128], lsb[64:128, 0:128])
                                wacc = mp.tile([128, 512], f32, tag="mm", name="warm")
                                nc.tensor.matmul(
                                    wacc[:], wu_s[:, 0:128], wu_s[:],
                                    start=True, stop=True,
                                )
                            rinv_t = rp.tile([128, 512], f32, tag="rinv", name="rinv")
                            nc.vector.reciprocal_approx_fast(rinv_t[:], lsb[:])
                            rinv = {h0: rinv_t[0:64, :], h1: rinv_t[64:128, :]}

                            for h in (h0, h1):
                                nc.vector.tensor_tensor(
                                    oc_s[h // 2][
                                        64 * (h % 2) : 64 * (h % 2) + 64, w0 : w0 + TQW
                                    ],
                                    o_t[h][0:64, :],
                                    rinv[h],
                                    mybir.AluOpType.mult,
                                )
                        prev_fin = fin
                        if g == 3 and hp == 0:
                            fm.add(proj3_a_units())
                # final segment normalize (with keep-warm matmuls inside)
                prev_fin()
                fm.drain()

            # ---- schedule ----
            drain(kq_units(0, fs=(0, 2)))
            attn_all()
            drain(proj3_b_units())

    nc.compile()
    return nc


def get_nc():
    if "nc" not in _nc_cache:
        _nc_cache["nc"] = _build_bass()
    return _nc_cache["nc"]


def _shard_inputs(x, W_kqv, W_proj):
    """Build the 8 per-core input maps: shard, transpose, cast to bf16 and
    pack 128-row panels along the free dim."""
    bf16 = ml_dtypes.bfloat16

    def pack(a):  # [128*k, n] -> [128, k*n], panel-major along free dim
        k = a.shape[0] // 128
        return np.ascontiguousarray(
            a.reshape(k, 128, a.shape[1]).transpose(1, 0, 2).reshape(128, -1)
        ).astype(bf16)

    f8 = ml_dtypes.float8_e4m3

    in_maps = []
    for core in range(8):
        b, hg = core // 4, core % 4
        heads = range(4 * hg, 4 * hg + 4)
        xt = x[b].T  # [C, T]
        # bf16 xt, window 0 only: [128, c*512 + t']
        xtw = xt.reshape(8, 128, 4, 512)  # [c, p, g, t']
        xtp = np.ascontiguousarray(xtw[:, :, 0].transpose(1, 0, 2).reshape(128, -1)).astype(bf16)
        # fp8 xt, windows 1-3, contraction pairs: [128, (g-1)*4096 + cp*1024 + par*512 + t']
        xtf = np.ascontiguousarray(
            xt.reshape(4, 2, 128, 4, 512)[:, :, :, 1:]  # [cp, par, p, g-1, t']
            .transpose(2, 3, 0, 1, 4)
            .reshape(128, -1)
        ).astype(f8)
        k_rows = [W_kqv[64 * h : 64 * (h + 1)] for h in heads]
        q_rows = [W_kqv[C + 64 * h : C + 64 * (h + 1)] for h in heads]
        v_rows = [W_kqv[2 * C + 64 * h : 2 * C + 64 * (h + 1)] for h in heads]
        # feature-tile order [k01, q01, k23, q23] (see FBLK in _build_bass)
        wqk_cat = np.concatenate(
            k_rows[0:2] + q_rows[0:2] + k_rows[2:4] + q_rows[2:4], 0
        )  # [512 feat, 1024 c]
        # f-major packing: [p, f*1024 + c*128 + fi]
        wqk = np.ascontiguousarray(
            wqk_cat.reshape(4, 128, 8, 128).transpose(3, 0, 2, 1).reshape(128, -1)
        ).astype(bf16)
        # fp8 SwInterleave packing: per (f, cp) block of 256, the physical
        # column sequence is [A_f127, B_f127, ..., A_f0, B_f0] where A/B are
        # the two contraction-pair halves (par) and features run reversed --
        # the layout the PE's DoubleRowSwInterleave weight load expects
        wqkf = np.ascontiguousarray(
            wqk_cat.reshape(4, 128, 4, 2, 128)[:, ::-1]  # [f, fi_rev, cp, par, ci]
            .transpose(4, 0, 2, 1, 3)                    # [ci, f, cp, fi_rev, par]
            .reshape(128, -1)
        ).astype(f8)
        wv_cat = np.concatenate(v_rows, 0).T  # [1024 c, 256]
        wv = pack(wv_cat)
        # fp8 pair packing: [p, cp*512 + par*256 + x]
        wvf = np.ascontiguousarray(
            wv_cat.reshape(4, 2, 128, 256).transpose(2, 0, 1, 3).reshape(128, -1)
        ).astype(f8)
        wp = pack(W_proj[:, 256 * hg : 256 * (hg + 1)].T)
        in_maps.append(
            {"xt": xtp, "xtf": xtf, "wqk": wqk, "wqkf": wqkf,
             "wv": wv, "wvf": wvf, "wp": wp,
             "eye": np.eye(128, dtype=bf16)}
        )
    return in_maps


def kernel(x, W_kqv, W_proj, b_proj):
    from concourse.bass_utils import run_bass_kernel_spmd

    x = np.asarray(x, dtype=np.float32)
    W_kqv = np.asarray(W_kqv, dtype=np.float32)
    W_proj = np.asarray(W_proj, dtype=np.float32)
    b_proj = np.asarray(b_proj, dtype=np.float32)
    nc = get_nc()
    in_maps = _shard_inputs(x, W_kqv, W_proj)
    res = run_bass_kernel_spmd(nc, in_maps, core_ids=list(range(8)))
    B = x.shape[0]
    out = np.empty((B, T, C), np.float32)
    for b in range(B):
        acc = res.results[4 * b]["y"].astype(np.float32).copy()
        for hg in range(1, 4):
            acc += res.results[4 * b + hg]["y"]
        out[b] = acc + b_proj[None, :]
    return out



# revision 56
# speedup vs baseline: 1.2253x; 1.0205x over previous
"""Causal self-attention kernel for 8 trn2 NeuronCores.

Sharding: core c handles batch b = c // 4 and local head group hg = c % 4
(4 of the 16 heads). Tensor-parallel over heads for kqv / attention and
row-parallel for the output projection; the 4 per-batch partial projections
(bf16) are summed on the host, where the bias is also added.

Inputs are pre-tiled on the host into [128, *] SBUF-ready layouts so every
load is a large multi-queue DMA. Feature tiles are packed [k01,q01,k23,q23]
so the first DMA chunk covers exactly what the first attention blocks need.

Device kernel (per core): one flat software-pipelined emitter over all
(window, head-pair) segments:
  - each pair's O-matmuls are emitted one pair late, and each segment's
    O-flush + normalize is emitted after the NEXT segment's first S/exp, so
    neither the PE queue nor ScalarE drains at hp/window boundaries (a
    drained PE queue also defeats the background LDWEIGHTS prefetch, which
    is what lets back-to-back matmuls sustain ~216ns/512cols)
  - dense work (later windows' k/q/v projections, earlier windows' output
    projections) lives in one global filler queue, popped between exp
    emissions proportionally to the remaining blocks; force-points drain a
    unit group before the instruction that reads its output is emitted
  - fp8e4: kq for windows 1-3 uses DoubleRowSwInterleave with host-packed
    interleaved+reversed weights (contiguous fast weight load, full 2x fp8
    column rate); v projections and attention O-matmuls use DoubleRow pair
    views; window 0 kq/v and the output projection stay bf16 (fp8 noise on
    short-context rows / in the projection reaches the output unaveraged
    and blows the 2e-2 budget - measured, not theoretical)
  - exp strips: S^T psum -> ScalarE exp -> fp8 [j0:h0|h1 | j1:h0|h1] tile;
    diagonal blocks use one 3D-AP activation covering both heads' [cs:512]
    strips (saves the 2nd instruction's ~350-cycle overhead); causal masks
    via GpSimd affine_select; denominators via ones-columns in the O
    stationary (psum rows 64:128 accumulate sum(P) for free)
  - tail: window 3's oc_s[0] projection half (proj3_a) runs as filler in
    the hp=1 exp shadow; after the last normalize (ScalarE lsb copies + a
    gated bf16 keep-warm matmul), proj3_b folds the bf16 a-half into psum
    with an identity matmul and stages out via ScalarE/DVE copies, each
    128-row y strip DMA'd (bf16) as soon as it completes

Per tq window g of 512 (pairs m <= 2g+1, causally trimmed):
  S^T = k^T.T q^T   (K=64; the head pair runs concurrently on PE row
                     groups 0-63/64-127 via base_partition tile_position)
  P = exp(S^T/8)    no max subtraction (scores are O(1))
  O^T psum[0:64] += [v_2m v_2m+1] @ [P_2m P_2m+1]   (fp8 DoubleRow)
  normalize: copies stack both heads' denominators in one tile, one
             reciprocal_approx_fast covers both, then multiply -> oc bf16
  proj: y[:, window] = O_cat^T.T @ Wp^T -> bf16 -> DMA
"""

import numpy as np
import ml_dtypes

T = 2048
C = 1024
NH_LOCAL = 4
D = 64
TQW = 512  # tq window width
NGRP = T // TQW  # 4 tq windows

_nc_cache = {}


def _build_bass():
    import concourse.mybir as mybir
    import concourse.tile as tile
    from concourse import bacc

    f32 = mybir.dt.float32
    bf16 = mybir.dt.bfloat16
    f8 = mybir.dt.float8e4

    nc = bacc.Bacc(None, target_bir_lowering=False)
    # pre-tiled inputs: [128, packed free dim] (see _shard_inputs)
    # bf16 copies feed window 0 (short-context rows need the precision);
    # fp8 pair-interleaved copies feed windows 1-3 via DoubleRow matmuls
    xt_d = nc.dram_tensor("xt", [128, 4096], bf16, kind="ExternalInput")
    xtf_d = nc.dram_tensor("xtf", [128, 3 * 4096], f8, kind="ExternalInput")
    wqk_d = nc.dram_tensor("wqk", [128, 8 * 512], bf16, kind="ExternalInput")
    wqkf_d = nc.dram_tensor("wqkf", [128, 4096], f8, kind="ExternalInput")
    wv_d = nc.dram_tensor("wv", [128, 8 * 256], bf16, kind="ExternalInput")
    wvf_d = nc.dram_tensor("wvf", [128, 2048], f8, kind="ExternalInput")
    wp_d = nc.dram_tensor("wp", [128, 2 * C], bf16, kind="ExternalInput")
    eye_d = nc.dram_tensor("eye", [128, 128], bf16, kind="ExternalInput")
    y_d = nc.dram_tensor("y", [T, C], bf16, kind="ExternalOutput")

    # host packs wqk/wqkf feature tiles in order [k01, q01, k23, q23] so the
    # first DMA chunk covers everything attn(0) hp=0 needs; f is the semantic
    # tile index (0=k01, 1=k23, 2=q01, 3=q23) used by kq_s / attn
    FBLK = {0: 0, 2: 1, 1: 2, 3: 3}

    with tile.TileContext(nc) as tc:
        with (
            tc.tile_pool(name="persist", bufs=1) as pp,
            tc.tile_pool(name="mmp", bufs=2, space="PSUM") as mp,
            tc.tile_pool(name="spsum", bufs=2, space="PSUM") as sp,
            tc.tile_pool(name="opsum", bufs=1, space="PSUM") as op,
            tc.tile_pool(name="ptp", bufs=4) as ptp,
            tc.tile_pool(name="rp", bufs=4) as rp,
            tc.tile_pool(name="ysb", bufs=3) as ysb,
        ):
            xt_a = pp.tile([128, 2048], bf16, tag="xta", name="xta")
            xt_b = pp.tile([128, 2048], bf16, tag="xtb", name="xtb")
            xtf_s = pp.tile([128, 3 * 4096], f8, tag="xtf", name="xtf")
            wqk_a = pp.tile([128, 2048], bf16, tag="wqka", name="wqka")
            wqk_b = pp.tile([128, 2048], bf16, tag="wqkb", name="wqkb")
            wqkf_s = pp.tile([128, 4096], f8, tag="wqkf", name="wqkf")
            wv_s = pp.tile([128, 8 * 256], bf16, tag="wv", name="wv")
            wvf_s = pp.tile([128, 2048], f8, tag="wvf", name="wvf")
            wp_s = pp.tile([128, 2 * C], bf16, tag="wp", name="wp")
            eye_s = pp.tile([128, 128], bf16, tag="eye", name="eye")
            kq_s = [pp.tile([128, T], bf16, tag=f"kq{f}", name=f"kq{f}") for f in range(4)]
            v_s = pp.tile([128, 4 * T], f8, tag="vall", name="vall")
            oc_s = [pp.tile([128, T], bf16, tag=f"oc{p}", name=f"oc{p}") for p in range(2)]
            wu_s = pp.tile([128, 512], bf16, tag="wu", name="wu")
            # bf16 copy of tk chunks j=0,1 (v | ones): short-context queries
            # (t<256) see little averaging, too noisy for the fp8 path
            vb_s = pp.tile([128, 1024], bf16, tag="vb", name="vb")
            # window-3 proj strips held in SBUF across the hp=0/hp=1 split
            ys3_s = [
                pp.tile([128, C], bf16, tag=f"ys3_{i}", name=f"ys3_{i}")
                for i in range(4)
            ]

            # slicing helpers for the packed layouts
            def xt_w(c):  # bf16 moving operand, window 0, contraction chunk c
                t = xt_a if c < 4 else xt_b
                return t[:, 512 * (c % 4) : 512 * (c % 4 + 1)]

            def xt_j(c, j):  # bf16 stationary for v chunks 0-3
                t = xt_a if c < 4 else xt_b
                o = 512 * (c % 4) + 128 * (j % 4)
                return t[:, o : o + 128]

            def wqk_blk(b):  # bf16 feature block b (packed order)
                t = wqk_a if b < 2 else wqk_b
                return t[:, 1024 * (b % 2) : 1024 * (b % 2 + 1)]

            def xtf_w(cp, g):  # fp8 [p,2,512] rhs pair view, windows 1-3
                o = 4096 * (g - 1) + 1024 * cp
                return xtf_s[:, o : o + 1024].rearrange(
                    "p (two x) -> p two x", two=2
                )

            def xtf_j(cp, j):  # fp8 [p,2,128] stationary pair view, chunks 4-15
                o = 4096 * (j // 4 - 1) + 1024 * cp
                t0 = 128 * (j % 4)
                return xtf_s[:, o : o + 1024].rearrange(
                    "p (two x) -> p two x", two=2
                )[:, :, t0 : t0 + 128]

            # ---- HAM warmup: PE busy from boot so real matmuls run warm ----
            # short: just enough to cover the first DMA chunk's latency; the
            # first kq chains continue the warm streak
            nc.gpsimd.memset(wu_s[:], 0.03125)
            for _ in range(12):
                wacc = mp.tile([128, 512], f32, tag="mm", name="warm")
                nc.tensor.matmul(wacc[:], wu_s[:, 0:128], wu_s[:], start=True, stop=True)

            # ones blocks for the denominator trick (v regions overwritten
            # later); contiguous memset on GpSimd - off Vector's critical path
            nc.gpsimd.memset(v_s[:], 1.0)
            nc.gpsimd.memset(vb_s[:], 1.0)

            # input DMAs: few large issues ordered by first use (a single
            # dma_start is split across all 16 queues by the framework)
            nc.sync.dma_start(wqk_a[:], wqk_d[:, 0:2048])
            nc.sync.dma_start(xt_a[:], xt_d[:, 0:2048])
            nc.sync.dma_start(xt_b[:], xt_d[:, 2048:4096])
            nc.sync.dma_start(wv_s[:], wv_d[:])
            nc.sync.dma_start(wqk_b[:], wqk_d[:, 2048:4096])
            nc.sync.dma_start(wqkf_s[:], wqkf_d[:])
            nc.sync.dma_start(xtf_s[:, 0:4096], xtf_d[:, 0:4096])
            nc.sync.dma_start(wvf_s[:], wvf_d[:])
            nc.sync.dma_start(wp_s[:], wp_d[:])
            nc.sync.dma_start(xtf_s[:, 4096:12288], xtf_d[:, 4096:12288])
            nc.sync.dma_start(eye_s[:], eye_d[:])

            # ---- dense work generators (emitted one instruction at a time) ----
            def kq_units(g, fs=(0, 1, 2, 3)):
                units = []
                for f in fs:
                    b = FBLK[f]
                    st = {}
                    if g == 0:
                        for c in range(8):
                            def mm(b=b, c=c, st=st):
                                if c == 0:
                                    st["acc"] = mp.tile([128, 512], f32, tag="mm", name="mmkq")
                                nc.tensor.matmul(
                                    st["acc"][:],
                                    wqk_blk(b)[:, 128 * c : 128 * (c + 1)],
                                    xt_w(c),
                                    start=(c == 0),
                                    stop=(c == 7),
                                )
                            units.append(mm)
                    else:
                        for cp_ in range(4):
                            def mm(b=b, cp_=cp_, g=g, st=st):
                                if cp_ == 0:
                                    st["acc"] = mp.tile([128, 512], f32, tag="mm", name="mmkq")
                                # SwInterleave: wqkf is host-packed with the
                                # A/B contraction pair interleaved per feature
                                # column (reversed), so LDWEIGHTS is one
                                # contiguous fast load and the fp8 matmul
                                # sustains its full 2x column rate
                                nc.tensor.matmul(
                                    st["acc"][:],
                                    wqkf_s[
                                        :, 1024 * b + 256 * cp_ : 1024 * b + 256 * (cp_ + 1)
                                    ],
                                    xtf_w(cp_, g),
                                    start=(cp_ == 0),
                                    stop=(cp_ == 3),
                                    perf_mode=mybir.MatmulPerfMode.DoubleRowSwInterleave,
                                )
                            units.append(mm)
                    def cp(f=f, g=g, st=st):
                        nc.vector.tensor_copy(
                            kq_s[f][:, TQW * g : TQW * (g + 1)], st["acc"][:]
                        )
                    units.append(cp)
                return units

            def v_units(g, js=None):
                units = []
                for j in js if js is not None else range(4 * g, 4 * g + 4):
                    st = {}
                    if g == 0:
                        for c in range(8):
                            def mm(j=j, c=c, st=st):
                                if c == 0:
                                    st["acc"] = mp.tile([128, 512], f32, tag="mm", name="mmv")
                                nc.tensor.matmul(
                                    st["acc"][:, :256],
                                    xt_j(c, j),
                                    wv_s[:, 256 * c : 256 * (c + 1)],
                                    start=(c == 0),
                                    stop=(c == 7),
                                )
                            units.append(mm)
                    else:
                        for cp_ in range(4):
                            def mm(j=j, cp_=cp_, st=st):
                                if cp_ == 0:
                                    st["acc"] = mp.tile([128, 512], f32, tag="mm", name="mmv")
                                nc.tensor.matmul(
                                    st["acc"][:, :256],
                                    xtf_j(cp_, j),
                                    wvf_s[:, 512 * cp_ : 512 * (cp_ + 1)].rearrange(
                                        "p (two x) -> p two x", two=2
                                    ),
                                    start=(cp_ == 0),
                                    stop=(cp_ == 3),
                                    perf_mode=mybir.MatmulPerfMode.DoubleRow,
                                )
                            units.append(mm)
                    def cp(j=j, st=st):
                        nc.vector.tensor_copy(
                            v_s[:, 512 * j : 512 * j + 512].rearrange(
                                "p (h x) -> p h x", h=4
                            )[:, :, 0:64],
                            st["acc"][:, 0:256].rearrange("p (h x) -> p h x", h=4),
                        )
                        if j < 2:
                            nc.vector.tensor_copy(
                                vb_s[:, 512 * j : 512 * (j + 1)].rearrange(
                                    "p (h x) -> p h x", h=4
                                )[:, :, 0:64],
                                st["acc"][:, 0:256].rearrange("p (h x) -> p h x", h=4),
                            )
                    units.append(cp)
                return units

            def proj_units(g):
                # windows 0..2: full projection, y strips DMA'd in pairs
                units = []
                for pi in range(2):
                    i0 = 4 * g + 2 * pi
                    st = {}
                    def alloc(st=st):
                        st["ys"] = ysb.tile([128, 2 * C], bf16, tag="ys", name="ys")
                    units.append(alloc)
                    for k in range(2):
                        i = i0 + k
                        for u in range(2):
                            for ci in range(2):
                                def mm(i=i, u=u, ci=ci, st=st):
                                    if ci == 0:
                                        st["acc"] = mp.tile([128, 512], f32, tag="mm", name="mmy")
                                    nc.tensor.matmul(
                                        st["acc"][:],
                                        oc_s[ci][:, 128 * i : 128 * (i + 1)],
                                        wp_s[:, 1024 * ci + 512 * u : 1024 * ci + 512 * (u + 1)],
                                        start=(ci == 0),
                                        stop=(ci == 1),
                                    )
                                units.append(mm)
                            def cp(k=k, u=u, st=st):
                                nc.vector.tensor_copy(
                                    st["ys"][:, 1024 * k + 512 * u : 1024 * k + 512 * (u + 1)],
                                    st["acc"][:],
                                )
                            units.append(cp)
                    def out(i0=i0, st=st):
                        nc.sync.dma_start(
                            y_d[128 * i0 : 128 * i0 + 256, :].rearrange(
                                "(two p) c -> p two c", two=2
                            ),
                            st["ys"][:].rearrange("p (two c) -> p two c", two=2),
                        )
                    units.append(out)
                return units

            def proj3_a_units():
                # window 3, oc_s[0] (head pair 0) half: filler for attn(3)
                # hp=1; partial y strips stream out early (bypass DMA)
                units = []
                for k, i in enumerate(range(12, 16)):
                    st = {}
                    for u in range(2):
                        def mm(i=i, u=u, st=st):
                            st["acc"] = mp.tile([128, 512], f32, tag="mm", name="mmy")
                            nc.tensor.matmul(
                                st["acc"][:],
                                oc_s[0][:, 128 * i : 128 * (i + 1)],
                                wp_s[:, 512 * u : 512 * (u + 1)],
                                start=True,
                                stop=True,
                            )
                        units.append(mm)
                        def cp(k=k, u=u, st=st):
                            nc.vector.tensor_copy(
                                ys3_s[k][:, 512 * u : 512 * (u + 1)], st["acc"][:]
                            )
                        units.append(cp)
                return units

            def proj3_b_units():
                # window 3, oc_s[1] half: the bf16 a-half is folded into the
                # psum with an identity matmul (PE), then the psum is staged
                # out with one ScalarE copy and one DVE copy per strip so no
                # single engine serializes the tail
                units = []
                for k, i in enumerate(range(12, 16)):
                    st = {}
                    def alloc(st=st):
                        st["yo"] = ysb.tile([128, C], bf16, tag="yo", name="yo")
                    units.append(alloc)
                    for u in range(2):
                        def mm(i=i, u=u, k=k, st=st):
                            # u1 accumulators borrow the (now idle) S-psum
                            # pool so each pool's ring is reused only every
                            # other strip -- the matmul then never WAR-waits
                            # on the previous strip's in-flight staging copy
                            if u == 0:
                                st["acc"] = mp.tile([128, 512], f32, tag="mm", name="mmy")
                            else:
                                st["acc"] = sp.tile([128, 2 * TQW], f32, tag="s", name="mmy")[:, 0:512]
                            nc.tensor.matmul(
                                st["acc"][:],
                                oc_s[1][:, 128 * i : 128 * (i + 1)],
                                wp_s[:, 1024 + 512 * u : 1024 + 512 * (u + 1)],
                                start=True,
                                stop=False,
                            )
                            nc.tensor.matmul(
                                st["acc"][:],
                                eye_s[:],
                                ys3_s[k][:, 512 * u : 512 * (u + 1)],
                                start=False,
                                stop=True,
                            )
                        units.append(mm)
                        def cp(u=u, st=st):
                            if u == 0:
                                nc.scalar.copy(st["yo"][:, 0:512], st["acc"][:])
                            else:
                                nc.vector.tensor_copy(st["yo"][:, 512:1024], st["acc"][:])
                        units.append(cp)
                    def out(i=i, st=st):
                        nc.sync.dma_start(y_d[128 * i : 128 * (i + 1), :], st["yo"][:])
                    units.append(out)
                return units

            def drain(units):
                for u in units:
                    u()
                units.clear()

            # ---- flat attention pipeline ----
            # One emitter for all (window, head-pair) segments. Emission is
            # software-pipelined so the PE queue never drains at a boundary:
            #   - each pair's O-matmuls are emitted one pair late (their exp
            #     has finished by the time they reach the queue head)
            #   - each segment's last-O flush + normalize is emitted after the
            #     NEXT segment's first S/exp, so ScalarE rolls straight on
            # Fillers live in one global queue with force-points for data
            # dependencies (a unit list must be fully emitted before the
            # instruction that reads its output is emitted).
            class FQ:
                def __init__(self):
                    self.q = []
                    self.added = 0
                    self.emitted = 0
                def add(self, units):
                    self.q += units
                    self.added += len(units)
                    return self.added  # mark
                def pop_some(self, n):
                    for _ in range(min(n, len(self.q))):
                        self.q.pop(0)()
                        self.emitted += 1
                def force_to(self, mark):
                    while self.emitted < mark and self.q:
                        self.q.pop(0)()
                        self.emitted += 1
                def drain(self):
                    self.pop_some(len(self.q))

            fm = FQ()
            marks = {}

            def attn_all():
                prev_fin = None
                nwin_blocks = [0]

                def pops(cap=6, dummy_ok=False):
                    avail = len(fm.q)
                    if avail > 0:
                        npop = min(cap, -(-avail // max(1, nwin_blocks[0])))
                        fm.pop_some(npop)

                for g in range(NGRP):
                    w0 = TQW * g
                    npairs = 2 * g + 2
                    nwin_blocks[0] = 2 * 2 * npairs
                    # window fillers (kq first: forced by next window's start)
                    if g == 0:
                        marks["v01"] = fm.add(v_units(0, js=(0, 1)))
                        marks["f13"] = fm.add(kq_units(0, fs=(1, 3)))
                        marks["v023"] = fm.add(v_units(0, js=(2, 3)))
                        marks["kq1"] = fm.add(kq_units(1))
                        marks["v1"] = fm.add(v_units(1))
                    elif g == 1:
                        marks["kq2"] = fm.add(kq_units(2))
                    elif g == 2:
                        # v2 spills from window 1 (only needed by pair m=4);
                        # proj1 moves to window 3, whose exp shadow has room
                        marks["v2"] = fm.add(v_units(2))
                        marks["kq3q"] = fm.add(kq_units(3, fs=(2, 3)))
                        marks["v3"] = fm.add(v_units(3))
                        fm.add(proj_units(0))
                    else:
                        marks["kq3k"] = fm.add(kq_units(3, fs=(0, 1)))
                        fm.add(proj_units(1) + proj_units(2))
                    if g > 0:
                        fm.force_to(marks[f"kq{g}"] if g < 3 else marks["kq3q"])
                    for hp in range(2):
                        h0, h1 = 2 * hp, 2 * hp + 1
                        if g == 0 and hp == 1:
                            fm.force_to(marks["f13"])
                        o_t = {}
                        pending_o = None
                        for m in range(npairs):
                            lowp = not (g == 0 and m == 0)
                            if lowp:
                                pt2 = ptp.tile([128, 4 * TQW], f8, tag="pt", name="pt")
                            else:
                                pt2 = ptp.tile([128, 4 * TQW], bf16, tag="ptb", name="ptb")
                            cs0 = max(0, 128 * (2 * m) - w0)
                            if g == 3 and m == 6:
                                fm.force_to(marks["kq3k"])
                            for p in range(2):
                                j = 2 * m + p
                                cs = max(0, 128 * j - w0)
                                s_t = sp.tile([128, 2 * TQW], f32, tag="s", name="s")
                                for idx, h in enumerate((h0, h1)):
                                    kT = kq_s[h // 2][64 * (h % 2) : 64 * (h % 2) + 64, :]
                                    qT = kq_s[2 + h // 2][64 * (h % 2) : 64 * (h % 2) + 64, :]
                                    nc.tensor.matmul(
                                        s_t[:, 512 * idx + cs : 512 * idx + 512],
                                        kT[:, 128 * j : 128 * (j + 1)],
                                        qT[:, w0 + cs : w0 + TQW],
                                        start=True,
                                        stop=True,
                                    )
                                po = 1024 * p
                                if cs:
                                    # diagonal: one 3D-AP activation covers both
                                    # heads' [cs:512] strips (saves the second
                                    # instruction's ~350-cycle overhead)
                                    nc.scalar.activation(
                                        pt2[:, po : po + 1024].rearrange(
                                            "p (two x) -> p two x", two=2
                                        )[:, :, cs:512],
                                        s_t[:].rearrange("p (two x) -> p two x", two=2)[
                                            :, :, cs:512
                                        ],
                                        mybir.ActivationFunctionType.Exp,
                                        scale=float(D) ** -0.5,
                                    )
                                else:
                                    nc.scalar.activation(
                                        pt2[:, po : po + 2 * TQW],
                                        s_t[:, 0 : 2 * TQW],
                                        mybir.ActivationFunctionType.Exp,
                                        scale=float(D) ** -0.5,
                                    )
                                # previous segment's O-flush + normalize must be
                                # emitted before any filler pops here: spilled
                                # proj/proj3a units read the oc it writes
                                if m == 0 and p == 0 and prev_fin is not None:
                                    prev_fin()
                                    prev_fin = None
                                # diagonal blocks have little exp shadow --
                                # defer filler to the next window's long exps
                                # (not in window 3: its spill would hit the tail)
                                pops(cap=10 if g == 0 else (2 if (cs and g < 3) else 6))
                                nwin_blocks[0] -= 1
                                if 128 * j >= w0:
                                    if p == 1 and cs > cs0:
                                        for idx in range(2):
                                            nc.gpsimd.memset(
                                                pt2[:, po + 512 * idx + cs0 : po + 512 * idx + cs],
                                                0.0,
                                            )
                                    for idx in range(2):
                                        nc.gpsimd.affine_select(
                                            out=pt2[:, po + 512 * idx + cs : po + 512 * idx + cs + 128],
                                            in_=pt2[:, po + 512 * idx + cs : po + 512 * idx + cs + 128],
                                            compare_op=mybir.AluOpType.is_ge,
                                            fill=0.0,
                                            base=0,
                                            pattern=[[1, 128]],
                                            channel_multiplier=-1,
                                        )
                            if pending_o is not None:
                                pending_o()
                            def o_emit(m=m, pt2=pt2, cs0=cs0, lowp=lowp, o_t=o_t,
                                       h0=h0, h1=h1, npairs=npairs, w0=w0, g=g):
                                if not o_t:
                                    o_t[h0] = op.tile([128, TQW], f32, tag="oh0", name="oh0")
                                    o_t[h1] = op.tile([128, TQW], f32, tag="oh1", name="oh1")
                                # v chunks 2m, 2m+1 must be resident
                                if g == 0:
                                    fm.force_to(marks["v01"] if m == 0 else marks["v023"])
                                elif m == 2 * g:
                                    fm.force_to(marks[f"v{g}"])
                                if lowp:
                                    for idx, h in enumerate((h0, h1)):
                                        vv = v_s[:].rearrange("p (jj x) -> p jj x", jj=16)[
                                            :, 2 * m : 2 * m + 2, 128 * h : 128 * (h + 1)
                                        ]
                                        rr = pt2[:].rearrange("p (jj x) -> p jj x", jj=2)[
                                            :, :, 512 * idx + cs0 : 512 * (idx + 1)
                                        ]
                                        nc.tensor.matmul(
                                            o_t[h][:, cs0:TQW],
                                            vv,
                                            rr,
                                            start=(m == 0),
                                            stop=(m == npairs - 1),
                                            perf_mode=mybir.MatmulPerfMode.DoubleRow,
                                        )
                                else:
                                    for p_ in range(2):
                                        jj = 2 * m + p_
                                        jcs = max(0, 128 * jj - w0)
                                        for idx, h in enumerate((h0, h1)):
                                            nc.tensor.matmul(
                                                o_t[h][:, jcs:TQW],
                                                vb_s[:, 512 * p_ + 128 * h : 512 * p_ + 128 * (h + 1)],
                                                pt2[:, 1024 * p_ + 512 * idx + jcs : 1024 * p_ + 512 * (idx + 1)],
                                                start=(jj == 0),
                                                stop=False,
                                            )
                            pending_o = o_emit
                        pops(cap=2)

                        def fin(pending_o=pending_o, o_t=o_t, h0=h0, h1=h1, w0=w0,
                                last=(g == NGRP - 1 and hp == 1)):
                            pending_o()
                            # both heads' denominators stacked in one tile so
                            # a single reciprocal instruction covers them
                            lsb = rp.tile([128, 512], f32, tag="lsb", name="lsb")
                            for idx, h in enumerate((h0, h1)):
                                if last:
                                    # tail: ScalarE is free, keep DVE for the
                                    # recip/mult/proj3b chain
                                    nc.scalar.copy(lsb[64 * idx : 64 * idx + 64, :], o_t[h][64:128, :])
                                else:
                                    nc.vector.tensor_copy(lsb[64 * idx : 64 * idx + 64, :], o_t[h][64:128, :])
                            if last:
                                # keep-warm: cast a strip of lsb into the wu
                                # tile (ScalarE, gated on the copies), then a
                                # full-rate bf16 matmul runs mid-normalize so
                                # HAM stays at 8/8 for the tail matmuls
                                nc.scalar.copy(wu_s[0:64, 0:128], lsb[64:128, 0:128])
                                wacc = mp.tile([128, 512], f32, tag="mm", name="warm")
                                nc.tensor.matmul(
                                    wacc[:], wu_s[:, 0:128], wu_s[:],
                                    start=True, stop=True,
                                )
                            rinv_t = rp.tile([128, 512], f32, tag="rinv", name="rinv")
                            nc.vector.reciprocal_approx_fast(rinv_t[:], lsb[:])
                            rinv = {h0: rinv_t[0:64, :], h1: rinv_t[64:128, :]}

                            for h in (h0, h1):
                                nc.vector.tensor_tensor(
                                    oc_s[h // 2][
                                        64 * (h % 2) : 64 * (h % 2) + 64, w0 : w0 + TQW
                                    ],
                                    o_t[h][0:64, :],
                                    rinv[h],
                                    mybir.AluOpType.mult,
                                )
                        prev_fin = fin
                        if g == 3 and hp == 0:
                            fm.add(proj3_a_units())
                # final segment normalize (with keep-warm matmuls inside)
                prev_fin()
                fm.drain()

            # ---- schedule ----
            drain(kq_units(0, fs=(0, 2)))
            attn_all()
            drain(proj3_b_units())

    nc.compile()
    return nc


def get_nc():
    if "nc" not in _nc_cache:
        _nc_cache["nc"] = _build_bass()
    return _nc_cache["nc"]


def _shard_inputs(x, W_kqv, W_proj):
    """Build the 8 per-core input maps: shard, transpose, cast to bf16 and
    pack 128-row panels along the free dim."""
    bf16 = ml_dtypes.bfloat16

    def pack(a):  # [128*k, n] -> [128, k*n], panel-major along free dim
        k = a.shape[0] // 128
        return np.ascontiguousarray(
            a.reshape(k, 128, a.shape[1]).transpose(1, 0, 2).reshape(128, -1)
        ).astype(bf16)

    f8 = ml_dtypes.float8_e4m3

    in_maps = []
    for core in range(8):
        b, hg = core // 4, core % 4
        heads = range(4 * hg, 4 * hg + 4)
        xt = x[b].T  # [C, T]
        # bf16 xt, window 0 only: [128, c*512 + t']
        xtw = xt.reshape(8, 128, 4, 512)  # [c, p, g, t']
        xtp = np.ascontiguousarray(xtw[:, :, 0].transpose(1, 0, 2).reshape(128, -1)).astype(bf16)
        # fp8 xt, windows 1-3, contraction pairs: [128, (g-1)*4096 + cp*1024 + par*512 + t']
        xtf = np.ascontiguousarray(
            xt.reshape(4, 2, 128, 4, 512)[:, :, :, 1:]  # [cp, par, p, g-1, t']
            .transpose(2, 3, 0, 1, 4)
            .reshape(128, -1)
        ).astype(f8)
        k_rows = [W_kqv[64 * h : 64 * (h + 1)] for h in heads]
        q_rows = [W_kqv[C + 64 * h : C + 64 * (h + 1)] for h in heads]
        v_rows = [W_kqv[2 * C + 64 * h : 2 * C + 64 * (h + 1)] for h in heads]
        # feature-tile order [k01, q01, k23, q23] (see FBLK in _build_bass)
        wqk_cat = np.concatenate(
            k_rows[0:2] + q_rows[0:2] + k_rows[2:4] + q_rows[2:4], 0
        )  # [512 feat, 1024 c]
        # f-major packing: [p, f*1024 + c*128 + fi]
        wqk = np.ascontiguousarray(
            wqk_cat.reshape(4, 128, 8, 128).transpose(3, 0, 2, 1).reshape(128, -1)
        ).astype(bf16)
        # fp8 SwInterleave packing: per (f, cp) block of 256, the physical
        # column sequence is [A_f127, B_f127, ..., A_f0, B_f0] where A/B are
        # the two contraction-pair halves (par) and features run reversed --
        # the layout the PE's DoubleRowSwInterleave weight load expects
        wqkf = np.ascontiguousarray(
            wqk_cat.reshape(4, 128, 4, 2, 128)[:, ::-1]  # [f, fi_rev, cp, par, ci]
            .transpose(4, 0, 2, 1, 3)                    # [ci, f, cp, fi_rev, par]
            .reshape(128, -1)
        ).astype(f8)
        wv_cat = np.concatenate(v_rows, 0).T  # [1024 c, 256]
        wv = pack(wv_cat)
        # fp8 pair packing: [p, cp*512 + par*256 + x]
        wvf = np.ascontiguousarray(
            wv_cat.reshape(4, 2, 128, 256).transpose(2, 0, 1, 3).reshape(128, -1)
        ).astype(f8)
        wp = pack(W_proj[:, 256 * hg : 256 * (hg + 1)].T)
        in_maps.append(
            {"xt": xtp, "xtf": xtf, "wqk": wqk, "wqkf": wqkf,
             "wv": wv, "wvf": wvf, "wp": wp,
             "eye": np.eye(128, dtype=bf16)}
        )
    return in_maps


def kernel(x, W_kqv, W_proj, b_proj):
    from concourse.bass_utils import run_bass_kernel_spmd

    x = np.asarray(x, dtype=np.float32)
    W_kqv = np.asarray(W_kqv, dtype=np.float32)
    W_proj = np.asarray(W_proj, dtype=np.float32)
    b_proj = np.asarray(b_proj, dtype=np.float32)
    nc = get_nc()
    in_maps = _shard_inputs(x, W_kqv, W_proj)
    res = run_bass_kernel_spmd(nc, in_maps, core_ids=list(range(8)))
    B = x.shape[0]
    out = np.empty((B, T, C), np.float32)
    for b in range(B):
        acc = res.results[4 * b]["y"].astype(np.float32).copy()
        for hg in range(1, 4):
            acc += res.results[4 * b + hg]["y"]
        out[b] = acc + b_proj[None, :]
    return out

